# revision 28
# baseline (speedup 1.0000x reference)
"""Trainium2 Bass kernel for BinarizeConv2dSDP.

Math (reference):
    s   = M + rv @ Z          (the rsqrt normalization is sign-preserving:
                               w = (m + rv@z) * rsqrt(...) with rsqrt > 0,
                               so sign(w) == sign(s))
    bw  = sign(s)             (O, I, 3, 3)
    ba  = sign(x)             (B, C, H, W)
    out = conv2d(ba, bw, stride 1, pad 1) * Alpha

Strategy:
    - Data-parallel over batch: 8 cores x 4 images each. M/Z/Alpha replicated.
    - All loads and stores ride the sync-engine hardware DGE queue (measured
      ~380 GB/s); wire order M, Z (z4 halved), x0 (halved), x1..x3, with
      output stores appended as their data becomes ready.
    - Weight synthesis s = M + sum_k rv_k Z_k on DVE in 2 column halves,
      paced by the Z stream; sign(s) on ACT; 2x9 PE transposes (64-channel
      chunks) into PSUM; per-tap 128-col pack copies on DVE pipeline behind
      the transposes, producing fp8 lhsT layouts.
    - Activations: sign(x) = +-1 fp8 in a zero-padded [128, 58 x 128] image
      holding copy A (cols 0..63) and copy B = A shifted left one column
      (cols 64..127, via DVE row-range copies). ACT signs in row quarters so
      conv tiles start as soon as their rows are ready.
    - Conv: 5 DoubleRow fp8 matmuls per 8-row output tile (all K=256):
      3 vertical (ky0,ky1) pairs at pair-stride 128 (one image row), plus 2
      ky2 horizontal pairs at pair-stride 64 (A->B = one-column shift):
      (kx0,kx1) and (kx2, zero-weight dummy). The DoubleRow pair stride must
      be 16B-aligned, which is exactly what copy B provides. PSUM f32, so
      conv sums are exact.
    - Evacuation on DVE (tensor_scalar by per-channel Alpha) to float16
      (conv values are integers, |conv| <= 1152 < 2048, exact in f16 up to
      the 2^-11 alpha-product rounding); f16 halves the store traffic and is
      upcast to f32 on the host.
"""

import os
import numpy as np

import concourse.bass as bass
import concourse.tile as tile
from concourse import bacc, mybir
from concourse.bass_utils import run_bass_kernel_spmd
from concourse.masks import make_identity

F32 = mybir.dt.float32
F16 = mybir.dt.float16
BF16 = mybir.dt.bfloat16
FP8 = mybir.dt.float8e4

USE_FP8 = bool(int(os.environ.get("BASS_KERNEL_FP8", "1")))
USE_OUT16 = bool(int(os.environ.get("BASS_KERNEL_OUT16", "1")))
USE_PAIR5 = bool(int(os.environ.get("BASS_KERNEL_PAIR5", "1")))
USE_XBYTE = False  # byte-gather DMA is descriptor-per-element: not viable

B_FULL = 32
N_CORES = 8
B_CORE = B_FULL // N_CORES  # 4 images per core
C = 128      # in channels
O = 128      # out channels
H = W = 56
HP = 58                       # padded rows
WP = 64                       # copy stride: B copy sits at col offset WP
WROW = 2 * WP                 # padded row pitch (A + shifted copy B)
KS = 3
NTAPS = KS * KS
IKK = C * NTAPS  # 1152
HKK = IKK // 2
ROWS_PER_TILE = 8            # output rows per PSUM tile -> N = 8*56 = 448
N_TILE = ROWS_PER_TILE * W   # 448 fp32 <= 512 (one PSUM bank)
N_ROW_TILES = H // ROWS_PER_TILE  # 7
ADT = FP8 if USE_FP8 else BF16
ODT = F16 if USE_OUT16 else F32

# sign/copy row quarters: pad-row ranges covering 0..57
BQ = [(0, 15), (15, 29), (29, 43), (43, 58)]


def _pair_ap(win, pair_stride):
    """Insert a [pair_stride, 2] dim after the partition dim of a 3D window
    AP — the k-tile (row-pair) feed for a DoubleRow matmul."""
    return bass.AP(
        win.tensor,
        win.offset,
        [list(win.ap[0]), [pair_stride, 2]] + [list(p) for p in win.ap[1:]],
    )


def build_program(rv: np.ndarray, n_img: int = B_CORE):
    """Build the per-core Bass program. rv values are baked as immediates."""
    nc = bacc.Bacc(
        "TRN2",
        target_bir_lowering=False,
        debug=False,
        num_devices=N_CORES,
    )

    x_t = nc.dram_tensor("x", (n_img, C, H, W), F32, kind="ExternalInput").ap()
    a_t = nc.dram_tensor("Alpha", (O, 1, 1), F32, kind="ExternalInput").ap()
    m_t = nc.dram_tensor("M", (O, C, KS, KS), F32, kind="ExternalInput").ap()
    z_t = nc.dram_tensor("Z", (5, O, C, KS, KS), F32, kind="ExternalInput").ap()
    out_t = nc.dram_tensor("out", (n_img, O, H, W), ODT, kind="ExternalOutput").ap()

    rv = np.asarray(rv, dtype=np.float32).reshape(-1)
    assert rv.shape[0] == 5

    with tile.TileContext(nc) as tc:
        with (
            tc.tile_pool(name="const", bufs=1) as const_pool,
            tc.tile_pool(name="wsyn", bufs=1) as wsyn_pool,
            tc.tile_pool(name="imgs", bufs=1) as img_pool,
            tc.tile_pool(name="ev", bufs=3) as ev_pool,
            tc.tile_pool(name="cpsum", bufs=6, space="PSUM") as cpsum_pool,
            tc.tile_pool(name="tpsum", bufs=1, space="PSUM") as tpsum_pool,
        ):
            # ---- input DMAs on the sync hw queue, in wire order ----
            identity = const_pool.tile([128, 128], BF16)
            make_identity(nc, identity)

            m_sb = wsyn_pool.tile([O, IKK], F32)
            nc.sync.dma_start(m_sb, m_t.rearrange("o i kh kw -> o (i kh kw)"))
            # all of Z in one [O, 5*IKK] tile, loaded in 4 DMAs (bigger
            # transfers run closer to wire rate; the tail is halved so the
            # last stt ops start as soon as their slice lands)
            QKK = IKK // 4
            zz = wsyn_pool.tile([O, 5 * IKK], F32)
            for k in range(4):
                nc.sync.dma_start(
                    zz[:, k * IKK : (k + 1) * IKK],
                    bass.AP(z_t.tensor, k * O * IKK, [[IKK, O], [1, IKK]]),
                )
            for h in range(4):
                nc.sync.dma_start(
                    zz[:, 4 * IKK + h * QKK : 4 * IKK + (h + 1) * QKK],
                    bass.AP(
                        z_t.tensor, 4 * O * IKK + h * QKK, [[IKK, O], [1, QKK]]
                    ),
                )

            alpha_sb = const_pool.tile([O, 1], F32)
            nc.sync.dma_start(alpha_sb, a_t.rearrange("o a b -> o (a b)"))

            xbs = [None] * n_img
            for img in range(n_img):
                xb = img_pool.tile([C, H * W], F32, name=f"xb{img}", tag=f"xb{img}")
                src = x_t[img].rearrange("c h w -> c (h w)")
                if img == 0:
                    hh = 28 * W
                    nc.sync.dma_start(xb[:, 0:hh], src[:, 0:hh])
                    nc.sync.dma_start(xb[:, hh:], src[:, hh:])
                else:
                    nc.sync.dma_start(xb, src)
                xbs[img] = xb

            # ---- padded activation buffers (A cols 0..63, B cols 64..127);
            # borders zeroed on Pool ----
            padded = []
            for img in range(n_img):
                pd = img_pool.tile(
                    [C, HP * WROW], ADT, name=f"pad{img}", tag=f"pad{img}"
                )
                pd3 = pd.rearrange("p (h w) -> p h w", w=WROW)
                nc.gpsimd.memset(pd3[:, 0, 0:62], 0.0)
                nc.gpsimd.memset(pd3[:, HP - 1, 0:62], 0.0)
                nc.gpsimd.memset(pd3[:, 1 : HP - 1, 0:1], 0.0)
                nc.gpsimd.memset(pd3[:, 1 : HP - 1, 57:59], 0.0)
                padded.append(pd3)

            if USE_FP8:
                bw_pair = wsyn_pool.tile([C, KS, 2, O], FP8)   # (kx, ky01)
                bw_pairf = bw_pair.rearrange("p a b o -> p (a b o)")
                bw_k2 = wsyn_pool.tile([C, 2, 2, O], FP8)      # ky2 kx pairs
                bw_k2f = bw_k2.rearrange("p a b o -> p (a b o)")
                if USE_PAIR5:
                    nc.gpsimd.memset(bw_k2f[:, 3 * O : 4 * O], 0.0)  # dummy
                else:
                    bw_single = wsyn_pool.tile([C, KS, O], FP8)
                tpP = tpsum_pool.tile([128, KS * 2 * O], BF16)
                tpS = tpsum_pool.tile([128, KS * O], BF16)
            else:
                bw_lhsT = wsyn_pool.tile([C, NTAPS, O], BF16)
                tpP = tpsum_pool.tile([128, 4 * O], BF16)
                tpS = tpsum_pool.tile([128, 5 * O], BF16)

            def sign_quarter(img, q):
                """ACT: sign x rows into pad copy A, quarter q."""
                r0, r1 = BQ[q]
                r0 = max(r0, 1)
                r1 = min(r1, 1 + H)
                pd3 = padded[img]
                dst = pd3[:, r0:r1, 1 : 1 + W]
                src = (
                    xbs[img][:, (r0 - 1) * W : (r1 - 1) * W]
                    .rearrange("c (h w) -> c h w", w=W)
                )
                nc.scalar.sign(dst, src)

            def copy_quarter(img, q):
                """DVE: pad copy B rows = copy A shifted one column left."""
                r0, r1 = BQ[q]
                pd3 = padded[img]
                nc.vector.tensor_copy(
                    pd3[:, r0:r1, WP : WP + 58], pd3[:, r0:r1, 1:59]
                )

            bw_nat = wsyn_pool.tile([O, IKK], BF16)
            # PE p-state warmup while the Z stream is on the wire: garbage
            # matmuls into the rotating conv PSUM tiles (results are
            # overwritten by each tile's start=True), done before sign(s)
            # needs bw_nat
            for _ in range(24):
                wt = cpsum_pool.tile([O, N_TILE], F32, tag="cv")
                nc.tensor.matmul(
                    wt, identity, bw_nat[:, 0:N_TILE], start=True, stop=True,
                    skip_group_check=True,
                )
            # then pace further warm matmuls on the arriving Z stream (f32,
            # 4 cyc/row) so the clock stays up until the transposes
            for k in range(5):
                nch = 2 if k < 4 else 4
                cw = IKK // nch
                for ic in range(nch):
                    woff = min(k * IKK + ic * cw, 5 * IKK - N_TILE)
                    wt = cpsum_pool.tile([O, N_TILE], F32, tag="cv")
                    nc.tensor.matmul(
                        wt,
                        m_sb[:, 0:128],
                        zz[:, woff : woff + N_TILE],
                        start=True,
                        stop=True,
                        skip_group_check=True,
                    )

            # ---- weight synthesis: s = M + sum_k rv_k Z_k, k-major in two
            # column halves so DVE paces behind the Z stream ----
            s_sb = wsyn_pool.tile([O, IKK], F32)
            bw3 = bw_nat.rearrange("o (i t) -> o i t", t=NTAPS)
            for k in range(5):
                nch = 2 if k < 4 else 4
                cw = IKK // nch
                for ic in range(nch):
                    csl = slice(ic * cw, (ic + 1) * cw)
                    nc.vector.scalar_tensor_tensor(
                        out=s_sb[:, csl],
                        in0=zz[:, k * IKK + ic * cw : k * IKK + (ic + 1) * cw],
                        scalar=float(rv[k]),
                        in1=(m_sb if k == 0 else s_sb)[:, csl],
                        op0=mybir.AluOpType.mult,
                        op1=mybir.AluOpType.add,
                    )
            alpha2 = const_pool.tile([O, 1], F32)
            nc.vector.tensor_scalar_mul(alpha2, alpha_sb, 1.0)

            # sign(s) on ACT in two halves; transposes in two 64-channel
            # chunks; per-tap pack copies on DVE pipeline behind them
            QW = IKK // 4
            for ic in range(4):
                csl = slice(ic * QW, (ic + 1) * QW)
                nc.scalar.sign(bw_nat[:, csl], s_sb[:, csl])
            CCH_T = 64
            # chunk B emitted in the conv's first-use tap order so the
            # first conv matmuls fire mid-pack
            for tc_i, order in enumerate(
                [list(range(NTAPS)), [0, 3, 1, 4, 2, 5, 6, 7, 8]]
            ):
                psl = slice(tc_i * CCH_T, (tc_i + 1) * CCH_T)
                for t in order:
                    ky, kx = divmod(t, KS)
                    if USE_FP8:
                        dst, toff = (
                            (tpP, (kx * 2 + ky) * O) if ky < 2 else (tpS, kx * O)
                        )
                    else:
                        dst, toff = (tpP, t * O) if t < 4 else (tpS, (t - 4) * O)
                    nc.tensor.transpose(
                        dst[psl, toff : toff + O],
                        bw3[:, psl, t],
                        identity,
                        tile_position=(0, tc_i * CCH_T),
                    )
            if USE_FP8:
                for i, t in enumerate([0, 3, 1, 4, 2, 5, 6, 7, 8]):
                    ky, kx = divmod(t, KS)
                    if ky < 2:
                        boff = (kx * 2 + ky) * O
                        srcf, dstf2, off = tpP, bw_pairf, boff
                    else:
                        dstf2 = bw_k2f if USE_PAIR5 else bw_single.rearrange(
                            "p a o -> p (a o)"
                        )
                        srcf, off = tpS, kx * O
                    if i % 2 == 0:
                        nc.vector.tensor_copy(
                            dstf2[:, off : off + O], srcf[:, off : off + O]
                        )
                    else:
                        nc.scalar.copy(
                            dstf2[:, off : off + O], srcf[:, off : off + O]
                        )
            else:
                nc.vector.tensor_copy(
                    bw_lhsT[:, 0:4, :], tpP.rearrange("p (t o) -> p t o", o=O)
                )
                nc.vector.tensor_copy(
                    bw_lhsT[:, 4:NTAPS, :], tpS.rearrange("p (t o) -> p t o", o=O)
                )

            # ---- image signs / B copies, quarter-granular ----
            need_b = USE_FP8 and USE_PAIR5
            for q in range(4):
                sign_quarter(0, q)
            if need_b:
                for q in range(4):
                    copy_quarter(0, q)
            if n_img > 1:
                for q in range(4):
                    sign_quarter(1, q)

            # ---- main conv loop ----
            for img in range(n_img):
                pd3 = padded[img]
                ev = None
                for nt in range(N_ROW_TILES):
                    half = 0 if nt < 4 else 1
                    idx = nt - 4 * half
                    if idx == 0:
                        ev = ev_pool.tile(
                            [O, (4 if half == 0 else 3) * N_TILE], ODT, tag="ev"
                        )
                    y0 = nt * ROWS_PER_TILE
                    cv = cpsum_pool.tile([O, N_TILE], F32, tag="cv")
                    if USE_FP8:
                        for kx in range(KS):
                            win0 = pd3[:, y0 : y0 + ROWS_PER_TILE, kx : kx + W]
                            nc.tensor.matmul(
                                cv,
                                bw_pair[:, kx],
                                _pair_ap(win0, WROW),
                                start=(kx == 0),
                                stop=False,
                                perf_mode=mybir.MatmulPerfMode.DoubleRow,
                            )
                        if USE_PAIR5:
                            for j in range(2):
                                win = pd3[
                                    :,
                                    y0 + 2 : y0 + 2 + ROWS_PER_TILE,
                                    2 * j : 2 * j + W,
                                ]
                                nc.tensor.matmul(
                                    cv,
                                    bw_k2[:, j],
                                    _pair_ap(win, WP),
                                    start=False,
                                    stop=(j == 1),
                                    perf_mode=mybir.MatmulPerfMode.DoubleRow,
                                )
                        else:
                            for kx in range(KS):
                                win = pd3[
                                    :,
                                    y0 + 2 : y0 + 2 + ROWS_PER_TILE,
                                    kx : kx + W,
                                ]
                                nc.tensor.matmul(
                                    cv,
                                    bw_single[:, kx],
                                    win,
                                    start=False,
                                    stop=(kx == KS - 1),
                                )
                    else:
                        t = 0
                        for ky in range(KS):
                            for kx in range(KS):
                                win = pd3[
                                    :,
                                    y0 + ky : y0 + ky + ROWS_PER_TILE,
                                    kx : kx + W,
                                ]
                                nc.tensor.matmul(
                                    cv,
                                    bw_lhsT[:, t, :],
                                    win,
                                    start=(t == 0),
                                    stop=(t == NTAPS - 1),
                                )
                                t += 1
                    dst = ev[:, idx * N_TILE : (idx + 1) * N_TILE]
                    nc.vector.tensor_scalar_mul(dst, cv, alpha2[:, 0:1])
                    if nt == 3:
                        # next image's B copies slot in behind this image's
                        # first evacs on DVE
                        if need_b and img + 1 < n_img:
                            for q in range(4):
                                copy_quarter(img + 1, q)
                        nc.sync.dma_start(
                            out_t[img, :, 0:32, :],
                            ev.rearrange("o (h w) -> o h w", w=W),
                        )
                    elif nt == 5 and img == n_img - 1:
                        nc.sync.dma_start(
                            out_t[img, :, 32:48, :],
                            ev[:, 0 : 2 * N_TILE].rearrange(
                                "o (h w) -> o h w", w=W
                            ),
                        )
                    elif nt == 6:
                        # image after next: signs on ACT
                        if img + 2 < n_img:
                            for q in range(4):
                                sign_quarter(img + 2, q)
                        if img == n_img - 1:
                            nc.sync.dma_start(
                                out_t[img, :, 48:56, :],
                                ev[:, 2 * N_TILE : 3 * N_TILE].rearrange(
                                    "o (h w) -> o h w", w=W
                                ),
                            )
                        else:
                            nc.sync.dma_start(
                                out_t[img, :, 32:56, :],
                                ev.rearrange("o (h w) -> o h w", w=W),
                            )

    nc.compile()
    return nc


def _ensure_ntff_hook():
    """Register the axon NTFF profiling hook if the image's antenv lacks it.

    Only used when BASS_KERNEL_TRACE=1 (dev profiling); best-effort.
    """
    import sys
    import types

    try:
        import antenv

        if hasattr(antenv, "axon_hooks"):
            return
        mod = types.ModuleType("antenv.axon_hooks")
        _hook = [None]
        mod.set_axon_ntff_profile_hook = lambda h: _hook.__setitem__(0, h)
        mod.get_axon_ntff_profile_hook = lambda: _hook[0]
        sys.modules["antenv.axon_hooks"] = mod
        antenv.axon_hooks = mod
        from trn_agent_boot.trn_boot import _ntff_profile_via_ctypes

        mod.set_axon_ntff_profile_hook(
            _ntff_profile_via_ctypes("/opt/axon/libaxon_pjrt.so")
        )
    except Exception as e:  # pragma: no cover - profiling is optional
        print(f"NTFF hook registration failed ({e}); tracing disabled")


def kernel(x, Alpha, M, Z, rv):
    x = np.ascontiguousarray(np.asarray(x, dtype=np.float32))
    Alpha = np.ascontiguousarray(np.asarray(Alpha, dtype=np.float32))
    M = np.ascontiguousarray(np.asarray(M, dtype=np.float32))
    Z = np.ascontiguousarray(np.asarray(Z, dtype=np.float32))
    rv = np.asarray(rv, dtype=np.float32)

    trace = bool(int(os.environ.get("BASS_KERNEL_TRACE", "0")))
    if trace:
        _ensure_ntff_hook()

    nc = build_program(rv)

    in_maps = []
    for c in range(N_CORES):
        in_maps.append(
            {
                "x": np.ascontiguousarray(x[c * B_CORE : (c + 1) * B_CORE]),
                "Alpha": Alpha,
                "M": M,
                "Z": Z,
            }
        )

    res = run_bass_kernel_spmd(
        nc,
        in_maps,
        core_ids=list(range(N_CORES)),
        trace=trace,
    )
    out = np.concatenate(
        [np.asarray(res.results[c]["out"], dtype=np.float32) for c in range(N_CORES)],
        axis=0,
    )
    if trace:
        kernel.last_results = res
    return out
